# revision 13
# baseline (speedup 1.0000x reference)
"""Trainium2 Bass kernel for a 2-layer LIF spiking network (T=50, B=1024,
784 -> 1024 -> 10), data-parallel over batch across 8 NeuronCores.

Strategy:
  - Layer-1 matmuls (x[t] @ W1.T) have no recurrent dependency: computed in
    bulk on the PE in a "transposed" formulation out = W1 @ x[t].T so the
    hidden dim lands on partitions and layer 2 needs no transpose.
  - fp32 accuracy via a 3-pass hi/lo split packed into ONE fp16 contraction
    stream of 2352 rows (padded to 19 k-tiles of 128):
        rows [xh; xl*2^11; xh]  x  [Wh; fp16(W*64/2^11); Wl]
    where xh = fp16(x), xl = fp16((x - xh)*2^11), Wh = fp16(W*64),
    Wl = fp16(W*64 - Wh).  PSUM accumulates all 19 tiles in fp32, yielding
    cur1*64 with ~2^-22 relative error.  The *64 scale keeps the W splits
    out of the fp16 subnormal range; LIF state is kept at 64x scale (exact,
    power of two) and the mem2 output is scaled back by 1/64 on write-out.
  - LIF updates run on the vector engine; layer-2 matmuls stack W2 hi/lo
    along the PE column (M) dim: lhsT [128, 20] = [W2h | W2l], so each
    spk1 h-tile streams once and PSUM rows 0-9 / 10-19 are summed after.
"""

import os
import sys

import numpy as np

sys.path.insert(0, "/opt/trn_rl_repo")

T, B, N_IN, N_HID, N_OUT = 50, 1024, 784, 1024, 10
NCORES = 8
BS = B // NCORES            # batch shard per core = 128
KT = 19                     # packed contraction tiles of 128 (2432 rows)
HT = N_HID // 128           # 8 hidden tiles
SCALE = 64.0
CHUNK = 4                   # timesteps per layer-2/psum chunk (N = 512)
SUPER = 8                   # timesteps per x-stream DMA window
NSUP = (T + SUPER - 1) // SUPER

LAST_RESULT = None          # BassKernelResults of the last run (for test.py)


def _build_bass(b1: float, b2: float):
    import concourse.bass as bass
    from concourse import bacc
    import concourse.mybir as mybir
    import concourse.tile as tile

    f32 = mybir.dt.float32
    f16 = mybir.dt.float16
    Alu = mybir.AluOpType
    Act = mybir.ActivationFunctionType

    nc = bacc.Bacc("TRN2", target_bir_lowering=False, debug=False,
                   num_devices=NCORES)

    TB = T * BS  # 6400
    xs_d = nc.dram_tensor("xs", [128, KT, TB], f16, kind="ExternalInput")
    w1_d = nc.dram_tensor("w1", [128, KT, HT, 128], f16, kind="ExternalInput")
    w2_d = nc.dram_tensor("w2", [128, HT, 32 + N_OUT], f16,
                          kind="ExternalInput")
    spk_d = nc.dram_tensor("spk2o", [N_OUT, TB], f32, kind="ExternalOutput")
    mem_d = nc.dram_tensor("mem2o", [N_OUT, TB], f32, kind="ExternalOutput")
    mark_d = nc.dram_tensor("mark", [128, 2], f32, kind="ExternalOutput")

    # supers: (t0, nsteps); first super small so the PE can start before
    # the full x window + weights have streamed in (~330 GB/s aggregate)
    supers = [(0, 4), (4, 8), (12, 8), (20, 8), (28, 8), (36, 8), (44, 6)]

    with tile.TileContext(nc) as tc:
        with (
            tc.tile_pool(name="const", bufs=1) as cpool,
            tc.tile_pool(name="xs", bufs=2) as xpool,
            tc.tile_pool(name="cur", bufs=2) as curpool,
            tc.tile_pool(name="spk", bufs=2) as spkpool,
            tc.tile_pool(name="state", bufs=1) as stpool,
            tc.tile_pool(name="outst", bufs=2) as opool,
            tc.tile_pool(name="ps1", bufs=5, space="PSUM") as ps1pool,
            tc.tile_pool(name="ps2", bufs=2, space="PSUM") as ps2pool,
        ):
            # ---- first x window + weights, split per-k so the first
            # (k-outer) matmul chunk's operands arrive progressively ----
            n_first = supers[0][1]
            xs0 = xpool.tile([128, KT, n_first * BS], f16, tag="xs")
            w1 = cpool.tile([128, KT, HT, 128], f16)
            w2 = cpool.tile([128, HT, 32 + N_OUT], f16)
            # warm up the PE p-state on a memset tile while the first
            # operands stream in (~3us of continuous busy => max clock)
            warm = cpool.tile([128, 512], f16)
            nc.gpsimd.memset(warm[:], 0.0)
            wps = ps1pool.tile([128, 512], f32, tag="p1", name="warmps")
            for r in range(26):
                nc.tensor.matmul(wps[:], warm[:, 0:128], warm[:],
                                 start=True, stop=True)
            # x / w1 in k-range pieces, ordered to match the k-outer
            # h-group consumption of the first chunk; few triggers per
            # queue so trigger-issue rate never gates the feed
            XPC = [(0, 4), (4, 8), (8, 12), (12, 16), (16, KT)]
            WPC = [(0, 6), (6, 12), (12, KT)]
            for a, b in WPC:
                nc.sync.dma_start(w1[:, a:b, 0:4], w1_d[:, a:b, 0:4])
            nc.sync.dma_start(w2[:], w2_d[:])
            for a, b in XPC:
                nc.gpsimd.dma_start(xs0[:, a:b, :],
                                    xs_d[:, a:b, 0:n_first * BS])
            for a, b in WPC:
                nc.gpsimd.dma_start(w1[:, a:b, 4:8], w1_d[:, a:b, 4:8])

            # ---- persistent LIF state (kept at 64x scale) ----
            m1 = stpool.tile([128, HT, 128], f32)   # mem1*64, free=(h, b)
            u1 = stpool.tile([128, HT, 128], f32)
            k1 = stpool.tile([128, HT, 128], f16)   # 1 - spike1 (keep mask)
            m2 = stpool.tile([N_OUT, 128], f32)
            u2 = stpool.tile([N_OUT, 128], f32)
            k2 = stpool.tile([N_OUT, 128], f16)
            nc.vector.memset(m1[:], 0.0)
            nc.vector.memset(k1[:], 1.0)
            nc.vector.memset(m2[:], 0.0)
            nc.vector.memset(k2[:], 1.0)

            for si, (s0, nsteps) in enumerate(supers):
                NW = nsteps * BS
                win0 = s0 * BS
                if si == 0:
                    xs = xs0
                else:
                    if si == 1:
                        # back-pressure marker: the sync queue may not issue
                        # bulk x transfers (which would steal DMA bandwidth
                        # from the startup-critical weights/x0) until chunk 0
                        # has consumed its first h-group (cur h=3 copied)
                        nc.sync.dma_start(mark_d[:], gate_cur[:, 3, 0:2])
                    # column halves on two trigger queues so chunk 0 can
                    # start on the first half while the second streams
                    xs = xpool.tile([128, KT, NW], f16, tag="xs")
                    nc.sync.dma_start(
                        xs[:, :, 0:NW // 2],
                        xs_d[:, :, win0:win0 + NW // 2])
                    nc.sync.dma_start(
                        xs[:, :, NW // 2:NW],
                        xs_d[:, :, win0 + NW // 2:win0 + NW])

                if si == len(supers) - 1:
                    sizes = [1] * nsteps            # taper: overlap LIF
                elif si == len(supers) - 2:
                    sizes = [4, 2, 1, 1]            # taper the LIF backlog
                else:
                    sizes = [CHUNK] * (nsteps // CHUNK)
                chunks = []
                c0 = 0
                for sz in sizes:
                    chunks.append((c0, sz))
                    c0 += sz

                # ---- layer-1 matmuls, chunk-major so the LIF chain can
                # start as soon as a chunk's last h-tile is evacuated ----
                cur = {}
                for ci, (c0, csz) in enumerate(chunks):
                    cur[ci] = curpool.tile([128, HT, csz * BS], f32,
                                           tag="cur1", name=f"cur1_{ci}")
                if si == 0:
                    gate_cur = cur[0]
                for ci, (c0, csz) in enumerate(chunks):
                    cs = slice(c0 * BS, (c0 + csz) * BS)
                    if si == 0 and ci == 0:
                        # k-outer over h-halves: consume each k-slice as it
                        # arrives from DRAM (cold-start pipelining)
                        for hg in (0, 4):
                            pss = [ps1pool.tile([128, csz * BS], f32,
                                                tag="p1",
                                                name=f"p1_{h}_{ci}")
                                   for h in range(hg, hg + 4)]
                            for k in range(KT):
                                for hi, h in enumerate(range(hg, hg + 4)):
                                    nc.tensor.matmul(
                                        pss[hi][:], w1[:, k, h, :],
                                        xs[:, k, cs],
                                        start=(k == 0), stop=(k == KT - 1))
                            for hi, h in enumerate(range(hg, hg + 4)):
                                nc.scalar.activation(cur[ci][:, h, :],
                                                     pss[hi][:], Act.Copy)
                    else:
                        for h in range(HT):
                            ps = ps1pool.tile([128, csz * BS], f32, tag="p1",
                                              name=f"p1_{h}_{ci}")
                            for k in range(KT):
                                nc.tensor.matmul(
                                    ps[:], w1[:, k, h, :], xs[:, k, cs],
                                    start=(k == 0), stop=(k == KT - 1))
                            nc.scalar.activation(cur[ci][:, h, :], ps[:],
                                                 Act.Copy)

                # ---- LIF1 + layer 2 + LIF2, per chunk ----
                for ci, (c0, csz) in enumerate(chunks):
                    NC_ = csz * BS
                    spk1 = spkpool.tile([128, HT, NC_], f16, tag="spk1")
                    for j in range(csz):
                        bs = slice(j * BS, (j + 1) * BS)
                        cj = cur[ci][:, :, bs]
                        # u = b1*m1 + cur ; m1' = u * keep ; spk/keep from m1'
                        nc.vector.scalar_tensor_tensor(
                            u1[:], m1[:], b1, cj, op0=Alu.mult, op1=Alu.add)
                        nc.vector.tensor_tensor(m1[:], u1[:], k1[:],
                                                op=Alu.mult)
                        nc.vector.tensor_scalar(
                            spk1[:, :, bs], m1[:], SCALE, None, op0=Alu.is_gt)
                        if s0 + c0 + j + 1 < T:
                            nc.vector.tensor_scalar(
                                k1[:], m1[:], SCALE, None, op0=Alu.is_le)
                    # layer 2: cur2.T = (W2*64) @ spk1 with hi/lo stacked on
                    # the M (output-partition) dim: each spk1 h-tile streams
                    # once; PSUM rows 0-9 = hi part, rows 10-19 = lo part.
                    p2 = ps2pool.tile([32 + N_OUT, NC_], f32, tag="p2")
                    for h in range(HT):
                        nc.tensor.matmul(
                            p2[:], w2[:, h, :], spk1[:, h, :],
                            start=(h == 0), stop=(h == HT - 1))
                    c2 = opool.tile([N_OUT, NC_], f32, tag="c2")
                    nc.scalar.activation(c2[:], p2[0:N_OUT, :], Act.Copy)
                    nc.vector.scalar_tensor_tensor(
                        c2[:], p2[32:32 + N_OUT, :], 1.0, c2[:],
                        op0=Alu.bypass, op1=Alu.add)
                    spk_st = opool.tile([N_OUT, NC_], f32, tag="spkst")
                    mem_st = opool.tile([N_OUT, NC_], f32, tag="memst")
                    for j in range(csz):
                        bs = slice(j * BS, (j + 1) * BS)
                        nc.vector.scalar_tensor_tensor(
                            u2[:], m2[:], b2, c2[:, bs], op0=Alu.mult,
                            op1=Alu.add)
                        nc.vector.tensor_tensor(m2[:], u2[:], k2[:],
                                                op=Alu.mult)
                        nc.vector.tensor_scalar(
                            spk_st[:, bs], m2[:], SCALE, None, op0=Alu.is_gt)
                        if s0 + c0 + j + 1 < T:
                            nc.vector.tensor_scalar(
                                k2[:], m2[:], SCALE, None, op0=Alu.is_le)
                        nc.vector.tensor_scalar(
                            mem_st[:, bs], m2[:], 1.0 / SCALE, None,
                            op0=Alu.mult)
                    ow = slice((s0 + c0) * BS, (s0 + c0 + csz) * BS)
                    nc.gpsimd.dma_start(spk_d[:, ow], spk_st[:])
                    nc.gpsimd.dma_start(mem_d[:, ow], mem_st[:])

    nc.compile()
    return nc


def _prep_inputs(x, W1, W2):
    """Host-side layout + hi/lo splits. Returns (per-core xs list, weights)."""
    f32 = np.float32
    f16 = np.float16
    x = np.asarray(x, f32)
    NPAD = KT * 128                                  # 2432 stream rows

    xh = x.astype(f16)                               # [T, B, 784]
    xl = ((x - xh.astype(f32)) * 2048.0).astype(f16)

    # stream rows [xh; xl*2^11; xh] -> S[2432, T, B]
    S = np.zeros((NPAD, T, B), f16)
    S[0:N_IN] = xh.transpose(2, 0, 1)
    S[N_IN:2 * N_IN] = xl.transpose(2, 0, 1)
    S[2 * N_IN:3 * N_IN] = S[0:N_IN]

    xs_cores = []
    for c in range(NCORES):
        Sc = S[:, :, c * BS:(c + 1) * BS]            # [2432, T, BS]
        Xc = Sc.reshape(KT, 128, T * BS).transpose(1, 0, 2)
        xs_cores.append(np.ascontiguousarray(Xc))    # [128, 19, 6400]

    W1s = np.asarray(W1, f32) * f32(SCALE)           # [N_HID, N_IN]
    W1T = np.ascontiguousarray(W1s.T)                # [784, 1024]
    wh = W1T.astype(f16)
    wl = (W1T - wh.astype(f32)).astype(f16)
    wp = (W1T / 2048.0).astype(f16)
    WS = np.zeros((NPAD, N_HID), f16)
    WS[0:N_IN] = wh
    WS[N_IN:2 * N_IN] = wp
    WS[2 * N_IN:3 * N_IN] = wl
    w1 = np.ascontiguousarray(
        WS.reshape(KT, 128, HT, 128).transpose(1, 0, 2, 3))  # [128,19,8,128]

    W2s = np.asarray(W2, f32) * f32(SCALE)           # [N_OUT, N_HID]
    W2T = np.ascontiguousarray(W2s.T)                # [1024, 10]
    w2h = W2T.astype(f16)
    w2l = (W2T - w2h.astype(f32)).astype(f16)
    w2 = np.zeros((128, HT, 32 + N_OUT), f16)
    w2[:, :, 0:N_OUT] = w2h.reshape(HT, 128, N_OUT).transpose(1, 0, 2)
    w2[:, :, 32:] = w2l.reshape(HT, 128, N_OUT).transpose(1, 0, 2)

    weights = {"w1": w1, "w2": w2}
    return xs_cores, weights


def _ensure_ntff_shim():
    """run_bass_kernel_spmd(trace) imports antenv.axon_hooks, absent in some
    images; install a graceful stand-in so tracing degrades instead of
    crashing."""
    try:
        import antenv.axon_hooks  # noqa: F401
        return
    except Exception:
        pass
    import types
    hook = None
    try:
        from trn_agent_boot.trn_boot import _ntff_profile_via_ctypes
        hook = _ntff_profile_via_ctypes("/opt/axon/libaxon_pjrt.so")
    except Exception:
        hook = None
    mod = types.ModuleType("antenv.axon_hooks")
    mod._hook = hook
    mod.get_axon_ntff_profile_hook = lambda: mod._hook
    mod.set_axon_ntff_profile_hook = lambda h: setattr(mod, "_hook", h)
    sys.modules["antenv.axon_hooks"] = mod


def kernel(x, W1, W2, beta1, beta2):
    global LAST_RESULT
    from concourse.bass_utils import run_bass_kernel_spmd

    _ensure_ntff_shim()

    b1 = float(np.clip(np.float32(beta1), 0.0, 1.0))
    b2 = float(np.clip(np.float32(beta2), 0.0, 1.0))

    xs_cores, weights = _prep_inputs(x, W1, W2)
    nc = _build_bass(b1, b2)

    in_maps = []
    for c in range(NCORES):
        m = {"xs": xs_cores[c]}
        m.update(weights)
        in_maps.append(m)

    res = run_bass_kernel_spmd(nc, in_maps, core_ids=list(range(NCORES)))
    LAST_RESULT = res

    spk_parts, mem_parts = [], []
    for c in range(NCORES):
        r = res.results[c]
        spk_parts.append(
            r["spk2o"].reshape(N_OUT, T, BS).transpose(1, 2, 0))
        mem_parts.append(
            r["mem2o"].reshape(N_OUT, T, BS).transpose(1, 2, 0))
    spk2 = np.ascontiguousarray(np.concatenate(spk_parts, axis=1))
    mem2 = np.ascontiguousarray(np.concatenate(mem_parts, axis=1))
    return spk2, mem2
